# revision 3
# baseline (speedup 1.0000x reference)
"""Contrast-depth MSE loss on 8 Trainium2 NeuronCores.

Math: with d = out - label (per image, 32x32 grid flattened to p in [0,1024)),
the loss is an exact quadratic form

    loss = sum_{p,q} C[p,q] * G[p,q] / (B*8*30*30),
    G[p,q] = sum_img d[img,p] * d[img,q]

where C (the contrast-depth-conv quadratic form) is supported on the
diagonals q-p in {0, +-1, +-31, +-32, +-33}.  Each core computes banded
Gram blocks G[128k+r, 128k+c] (c in [0,161)) on the TensorEngine with
PSUM accumulation over its 2048-image shard; the host applies the C
weights to the diagonals and reduces across cores.

DMA-engine load balancing: HBM->SBUF traffic is striped over 16 SDMA
engines by a fixed partition->engine map (engine 2k+1 serves partitions
{64+4k..67+4k, 96+4k..99+4k}).  Engine 15 is measurably ~20% slower than
its peers, so a uniform 16-images-per-partition layout makes it the
critical path.  The image layout is therefore ragged: partitions 92-95
and 124-127 (engine 15's) hold 13 images, partitions 0-23 hold 17, the
rest 16.  Slots that a partition doesn't fill are zero in d and fall out
of the Gram exactly.
"""

import numpy as np

_B = 16384
_H = 32
_W = 32
_P = _H * _W  # 1024 pixels
_NCORES = 8
_BSH = _B // _NCORES  # 2048 images per core
_TILE = 128
_BAND = 161  # 128 + max diagonal offset (33)

# Ragged slot layout (images per SBUF partition / DRAM row)
_NSLOT = 17  # max slots per row
_FREE = _NSLOT * _P  # 17408 f32 per partition
# row groups: [row0, row1, n_images]
_ROWGROUPS = [
    (0, 24, 17),
    (24, 92, 16),
    (92, 96, 13),  # DMA engine 15
    (96, 124, 16),
    (124, 128, 13),  # DMA engine 15
]
assert sum((r1 - r0) * n for r0, r1, n in _ROWGROUPS) == _BSH


def _block_ncols(k: int) -> int:
    return min(_BAND, _P - 128 * k)


_GRAM_COLS = sum(_block_ncols(k) for k in range(8))  # 7*161 + 128 = 1255


def _build_weights() -> np.ndarray:
    """[128, _GRAM_COLS] weights s.t. loss_sum = sum(W * gram_blocks)."""
    C = np.zeros((_P, _P), dtype=np.float64)
    offs = [(a, b) for a in range(3) for b in range(3) if (a, b) != (1, 1)]
    for a, b in offs:
        for i in range(_H - 2):
            for j in range(_W - 2):
                p = (i + a) * _W + (j + b)  # neighbor pixel
                q = (i + 1) * _W + (j + 1)  # center pixel
                C[p, p] += 1.0
                C[q, q] += 1.0
                C[p, q] -= 1.0
                C[q, p] -= 1.0
    W = np.zeros((_TILE, _GRAM_COLS), dtype=np.float64)
    off = 0
    for k in range(8):
        ncols = _block_ncols(k)
        for delta in (0, 1, 31, 32, 33):
            for r in range(_TILE):
                p = 128 * k + r
                q = p + delta
                c = r + delta
                if q >= _P or c >= ncols:
                    continue
                W[r, off + c] = C[p, q] * (1.0 if delta == 0 else 2.0)
        off += ncols
    return W


_WFULL = _build_weights()

_NC_CACHE = None


def _pack_shard(a: np.ndarray) -> np.ndarray:
    """[2048, 1024] f32 -> ragged-packed [128, _FREE]; pad cols unread."""
    Pk = np.empty((_TILE, _FREE), dtype=np.float32)
    img = 0
    for r0, r1, n in _ROWGROUPS:
        cnt = (r1 - r0) * n
        Pk[r0:r1, : n * _P] = a[img : img + cnt].reshape(r1 - r0, n * _P)
        img += cnt
    return Pk


def _build_nc():
    import concourse.bacc as bacc
    import concourse.mybir as mybir
    import concourse.tile as tile

    nc = bacc.Bacc()
    out_d = nc.dram_tensor("out", [_TILE, _FREE], mybir.dt.float32, kind="ExternalInput")
    lab_d = nc.dram_tensor("label", [_TILE, _FREE], mybir.dt.float32, kind="ExternalInput")
    gram_d = nc.dram_tensor(
        "gram", [_TILE, _GRAM_COLS], mybir.dt.float32, kind="ExternalOutput"
    )

    # DMA chunks: (row0, row1, col0, col1, slots-completed-after-this-chunk)
    # Slots 0-12 are full-width; 13-15 skip engine-15 rows; 16 is rows 0-24.
    _CHUNKS = [
        (0, 128, 0, 4 * _P, (0, 1, 2, 3)),
        (0, 128, 4 * _P, 8 * _P, (4, 5, 6, 7)),
        (0, 128, 8 * _P, 11 * _P, (8, 9, 10)),
        (0, 128, 11 * _P, 13 * _P, (11, 12)),
        (0, 92, 13 * _P, 16 * _P, ()),
        (96, 124, 13 * _P, 16 * _P, (13, 14, 15)),
        (0, 24, 16 * _P, 17 * _P, (16,)),
    ]
    # per-slot row spans holding real data (rest of d is zero-padded)
    def _slot_rows(tt):
        if tt < 13:
            return [(0, 128)]
        if tt < 16:
            return [(0, 92), (96, 124)]
        return [(0, 24)]

    with tile.TileContext(nc) as tc:
        with (
            tc.tile_pool(name="buf", bufs=1) as buf_pool,
            tc.tile_pool(name="ps", bufs=1, space="PSUM") as psum_pool,
        ):
            grams = []
            offs = []
            off = 0
            for k in range(8):
                ncols = _block_ncols(k)
                grams.append(
                    psum_pool.tile(
                        [_TILE, ncols], mybir.dt.float32, tag=f"g{k}", name=f"g{k}"
                    )
                )
                offs.append(off)
                off += ncols

            # persistent SBUF buffers: every chunk DMA can enqueue
            # immediately; no pool-slot rotation ever blocks the DMA stream.
            o = buf_pool.tile([_TILE, _FREE], mybir.dt.float32, tag="o", name="o")
            lb = buf_pool.tile([_TILE, _FREE], mybir.dt.float32, tag="l", name="l")
            d = buf_pool.tile([_TILE, _FREE], mybir.dt.bfloat16, tag="d", name="d")

            # zero d's ragged pad regions once, with quadrant-aligned
            # partition starts (0/32/64/96); later subs overwrite the
            # real-data parts of these spans.
            nc.gpsimd.memset(d[64:128, 13 * _P : _FREE], 0.0)
            nc.gpsimd.memset(d[0:64, 16 * _P : _FREE], 0.0)

            nslots_done = 0
            for r0, r1, c0, c1, slots in _CHUNKS:
                nc.sync.dma_start(out=o[r0:r1, c0:c1], in_=out_d[r0:r1, c0:c1])
                nc.scalar.dma_start(out=lb[r0:r1, c0:c1], in_=lab_d[r0:r1, c0:c1])
                for tt in slots:
                    base = tt * _P
                    for sr0, sr1 in _slot_rows(tt):
                        nc.vector.tensor_sub(
                            out=d[sr0:sr1, base : base + _P],
                            in0=o[sr0:sr1, base : base + _P],
                            in1=lb[sr0:sr1, base : base + _P],
                        )
                    first = nslots_done == 0
                    last = nslots_done == _NSLOT - 1
                    for k in range(8):
                        ncols = _block_ncols(k)
                        nc.tensor.matmul(
                            grams[k][:, :ncols],
                            lhsT=d[:, base + 128 * k : base + 128 * k + 128],
                            rhs=d[:, base + 128 * k : base + 128 * k + ncols],
                            start=first,
                            stop=last,
                        )
                    nslots_done += 1
            assert nslots_done == _NSLOT

            result = buf_pool.tile(
                [_TILE, _GRAM_COLS], mybir.dt.float32, tag="r", name="r"
            )
            # two half outputs so the first DMA overlaps the tail copies
            for k in range(4):
                nc.vector.tensor_copy(
                    out=result[:, offs[k] : offs[k] + _block_ncols(k)], in_=grams[k][:]
                )
            nc.sync.dma_start(out=gram_d[:, : offs[4]], in_=result[:, : offs[4]])
            for k in range(4, 8):
                nc.vector.tensor_copy(
                    out=result[:, offs[k] : offs[k] + _block_ncols(k)], in_=grams[k][:]
                )
            nc.scalar.dma_start(out=gram_d[:, offs[4] :], in_=result[:, offs[4] :])
    nc.finalize()
    return nc


def _run(out, label, trace=False):
    from concourse.bass_utils import run_bass_kernel_spmd

    global _NC_CACHE
    out = np.ascontiguousarray(np.asarray(out), dtype=np.float32).reshape(_B, _P)
    label = np.ascontiguousarray(np.asarray(label), dtype=np.float32).reshape(_B, _P)
    if _NC_CACHE is None:
        _NC_CACHE = _build_nc()
    in_maps = [
        {
            "out": _pack_shard(out[i * _BSH : (i + 1) * _BSH]),
            "label": _pack_shard(label[i * _BSH : (i + 1) * _BSH]),
        }
        for i in range(_NCORES)
    ]
    res = run_bass_kernel_spmd(
        _NC_CACHE, in_maps, core_ids=list(range(_NCORES)), trace=trace
    )
    total = 0.0
    for r in res.results:
        total += float((_WFULL * r["gram"].astype(np.float64)).sum())
    loss = total / (_B * 8 * (_H - 2) * (_W - 2))
    return np.asarray(np.float32(loss)), res


def kernel(out, label):
    loss, _ = _run(out, label, trace=False)
    return loss


# revision 6
# speedup vs baseline: 1.2295x; 1.2295x over previous
"""Contrast-depth MSE loss on 8 Trainium2 NeuronCores.

Math: with d = out - label (per image, 32x32 grid flattened to p in [0,1024)),
the loss is an exact quadratic form

    loss = sum_{p,q} C[p,q] * G[p,q] / (B*8*30*30),
    G[p,q] = sum_img d[img,p] * d[img,q]

where C (the contrast-depth-conv quadratic form) is supported on the
diagonals q-p in {0, +-1, +-31, +-32, +-33}.  Each core computes banded
Gram blocks G[128k+r, 128k+c] (c in [0,161)) on the TensorEngine with
PSUM accumulation over its 2048-image shard; the host applies the C
weights to the diagonals and reduces across cores.

DMA-engine load balancing: HBM->SBUF traffic is striped over 16 SDMA
engines.  Full-128-partition transfers give each engine 8 partitions'
bytes; an N-partition transfer is instead dealt in k = (largest divisor
of N <= 16) contiguous groups to engines 0..k-1.  Engine 15 runs ~20%
slower than its peers (port contention), so the image layout is ragged:
all 128 partitions hold 13 images (full-width transfers), partitions
0-119 hold 3 more (120-row transfers -> engines 0-14 only), and
partitions 0-23 hold one more (24-row transfer -> engines 0-11).  Slots
a partition doesn't fill are zero in d and fall out of the Gram exactly.
"""

import numpy as np

_B = 16384
_H = 32
_W = 32
_P = _H * _W  # 1024 pixels
_NCORES = 8
_BSH = _B // _NCORES  # 2048 images per core
_TILE = 128
_BAND = 161  # 128 + max diagonal offset (33)

# Ragged slot layout (images per SBUF partition / DRAM row)
_NSLOT = 17  # max slots per row
_FREE = _NSLOT * _P  # 17408 f32 per partition
# row groups: [row0, row1, n_images]; 24*17 + 96*16 + 8*13 = 2048
_ROWGROUPS = [
    (0, 24, 17),
    (24, 120, 16),
    (120, 128, 13),
]
assert sum((r1 - r0) * n for r0, r1, n in _ROWGROUPS) == _BSH


def _block_ncols(k: int) -> int:
    return min(_BAND, _P - 128 * k)


_GRAM_COLS = sum(_block_ncols(k) for k in range(8))  # 7*161 + 128 = 1255


def _build_weights() -> np.ndarray:
    """[128, _GRAM_COLS] weights s.t. loss_sum = sum(W * gram_blocks)."""
    C = np.zeros((_P, _P), dtype=np.float64)
    offs = [(a, b) for a in range(3) for b in range(3) if (a, b) != (1, 1)]
    for a, b in offs:
        for i in range(_H - 2):
            for j in range(_W - 2):
                p = (i + a) * _W + (j + b)  # neighbor pixel
                q = (i + 1) * _W + (j + 1)  # center pixel
                C[p, p] += 1.0
                C[q, q] += 1.0
                C[p, q] -= 1.0
                C[q, p] -= 1.0
    W = np.zeros((_TILE, _GRAM_COLS), dtype=np.float64)
    off = 0
    for k in range(8):
        ncols = _block_ncols(k)
        for delta in (0, 1, 31, 32, 33):
            for r in range(_TILE):
                p = 128 * k + r
                q = p + delta
                c = r + delta
                if q >= _P or c >= ncols:
                    continue
                W[r, off + c] = C[p, q] * (1.0 if delta == 0 else 2.0)
        off += ncols
    return W


_WFULL = _build_weights()

_NC_CACHE = None


def _pack_shard(a: np.ndarray) -> np.ndarray:
    """[2048, 1024] f32 -> ragged-packed [128, _FREE]; pad cols unread."""
    Pk = np.empty((_TILE, _FREE), dtype=np.float32)
    img = 0
    for r0, r1, n in _ROWGROUPS:
        cnt = (r1 - r0) * n
        Pk[r0:r1, : n * _P] = a[img : img + cnt].reshape(r1 - r0, n * _P)
        img += cnt
    return Pk


def _build_nc():
    import concourse.bacc as bacc
    import concourse.mybir as mybir
    import concourse.tile as tile

    nc = bacc.Bacc()
    out_d = nc.dram_tensor("out", [_TILE, _FREE], mybir.dt.float32, kind="ExternalInput")
    lab_d = nc.dram_tensor("label", [_TILE, _FREE], mybir.dt.float32, kind="ExternalInput")
    gram_d = nc.dram_tensor(
        "gram", [_TILE, _GRAM_COLS], mybir.dt.float32, kind="ExternalOutput"
    )

    # DMA chunks: (row0, row1, col0, col1, slots-completed-after-this-chunk)
    # Partial-width transfers go FIRST: their strided small-descriptor reads
    # are less HBM-efficient, and front-loading them keeps the stream tail on
    # full-width 16KB descriptors while their compute happens early.
    _CHUNKS = [
        (0, 120, 13 * _P, 16 * _P, (13, 14, 15)),
        (0, 24, 16 * _P, 17 * _P, (16,)),
        (0, 128, 0, 4 * _P, (0, 1, 2, 3)),
        (0, 128, 4 * _P, 8 * _P, (4, 5, 6, 7)),
        (0, 128, 8 * _P, 11 * _P, (8, 9, 10)),
        (0, 128, 11 * _P, 13 * _P, (11, 12)),
    ]

    def _slot_rows(tt):
        if tt < 13:
            return 128
        if tt < 16:
            return 120
        return 24

    with tile.TileContext(nc) as tc:
        with (
            tc.tile_pool(name="buf", bufs=1) as buf_pool,
            tc.tile_pool(name="ps", bufs=1, space="PSUM") as psum_pool,
        ):
            grams = []
            offs = []
            off = 0
            for k in range(8):
                ncols = _block_ncols(k)
                grams.append(
                    psum_pool.tile(
                        [_TILE, ncols], mybir.dt.float32, tag=f"g{k}", name=f"g{k}"
                    )
                )
                offs.append(off)
                off += ncols

            # persistent SBUF buffers: every chunk DMA can enqueue
            # immediately; no pool-slot rotation ever blocks the DMA stream.
            o = buf_pool.tile([_TILE, _FREE], mybir.dt.float32, tag="o", name="o")
            lb = buf_pool.tile([_TILE, _FREE], mybir.dt.float32, tag="l", name="l")
            d = buf_pool.tile([_TILE, _FREE], mybir.dt.bfloat16, tag="d", name="d")

            # zero d's ragged pad regions once, with quadrant-aligned
            # partition starts; later subs overwrite the real-data parts.
            nc.gpsimd.memset(d[96:128, 13 * _P : _FREE], 0.0)
            nc.gpsimd.memset(d[0:96, 16 * _P : _FREE], 0.0)

            def mm(k, base, first, last):
                ncols = _block_ncols(k)
                nc.tensor.matmul(
                    grams[k][:, :ncols],
                    lhsT=d[:, base + 128 * k : base + 128 * k + 128],
                    rhs=d[:, base + 128 * k : base + 128 * k + ncols],
                    start=first,
                    stop=last,
                )

            nslots_done = 0
            for r0, r1, c0, c1, slots in _CHUNKS:
                nc.sync.dma_start(out=o[r0:r1, c0:c1], in_=out_d[r0:r1, c0:c1])
                nc.scalar.dma_start(out=lb[r0:r1, c0:c1], in_=lab_d[r0:r1, c0:c1])
                for tt in slots:
                    base = tt * _P
                    nrow = _slot_rows(tt)
                    first = nslots_done == 0
                    last = nslots_done == _NSLOT - 1
                    if not last:
                        nc.vector.tensor_sub(
                            out=d[0:nrow, base : base + _P],
                            in0=o[0:nrow, base : base + _P],
                            in1=lb[0:nrow, base : base + _P],
                        )
                        for k in range(8):
                            mm(k, base, first, last)
                    else:
                        # final slot: high column half first so blocks 4-7
                        # stop early and their PSUM copies + output DMA
                        # overlap the low half's sub and matmuls
                        for h0, h1 in ((512, _P), (0, 512)):
                            nc.vector.tensor_sub(
                                out=d[0:nrow, base + h0 : base + h1],
                                in0=o[0:nrow, base + h0 : base + h1],
                                in1=lb[0:nrow, base + h0 : base + h1],
                            )
                        for k in (4, 5, 6, 7):
                            mm(k, base, first, last)
                    nslots_done += 1
            assert nslots_done == _NSLOT

            result = buf_pool.tile(
                [_TILE, _GRAM_COLS], mybir.dt.float32, tag="r", name="r"
            )
            base12 = 12 * _P
            for k in (4, 5, 6, 7):
                nc.vector.tensor_copy(
                    out=result[:, offs[k] : offs[k] + _block_ncols(k)], in_=grams[k][:]
                )
            # output rows split 120/8 so engine 15 carries no output bytes
            nc.scalar.dma_start(
                out=gram_d[0:120, offs[4] :], in_=result[0:120, offs[4] :]
            )
            nc.scalar.dma_start(
                out=gram_d[120:128, offs[4] :], in_=result[120:128, offs[4] :]
            )
            for k in (0, 1, 2, 3):
                mm(k, base12, False, True)
            for k in (0, 1, 2, 3):
                nc.vector.tensor_copy(
                    out=result[:, offs[k] : offs[k] + _block_ncols(k)], in_=grams[k][:]
                )
            nc.sync.dma_start(
                out=gram_d[0:120, : offs[4]], in_=result[0:120, : offs[4]]
            )
            nc.sync.dma_start(
                out=gram_d[120:128, : offs[4]], in_=result[120:128, : offs[4]]
            )
    nc.finalize()
    return nc


def _run(out, label, trace=False):
    from concourse.bass_utils import run_bass_kernel_spmd

    global _NC_CACHE
    out = np.ascontiguousarray(np.asarray(out), dtype=np.float32).reshape(_B, _P)
    label = np.ascontiguousarray(np.asarray(label), dtype=np.float32).reshape(_B, _P)
    if _NC_CACHE is None:
        _NC_CACHE = _build_nc()
    in_maps = [
        {
            "out": _pack_shard(out[i * _BSH : (i + 1) * _BSH]),
            "label": _pack_shard(label[i * _BSH : (i + 1) * _BSH]),
        }
        for i in range(_NCORES)
    ]
    res = run_bass_kernel_spmd(
        _NC_CACHE, in_maps, core_ids=list(range(_NCORES)), trace=trace
    )
    total = 0.0
    for r in res.results:
        total += float((_WFULL * r["gram"].astype(np.float64)).sum())
    loss = total / (_B * 8 * (_H - 2) * (_W - 2))
    return np.asarray(np.float32(loss)), res


def kernel(out, label):
    loss, _ = _run(out, label, trace=False)
    return loss


# revision 7
# speedup vs baseline: 1.2658x; 1.0295x over previous
"""Contrast-depth MSE loss on 8 Trainium2 NeuronCores.

Math: with d = out - label (per image, 32x32 grid flattened to p in [0,1024)),
the loss is an exact quadratic form

    loss = sum_{p,q} C[p,q] * G[p,q] / (B*8*30*30),
    G[p,q] = sum_img d[img,p] * d[img,q]

where C (the contrast-depth-conv quadratic form) is supported on the
diagonals q-p in {0, +-1, +-31, +-32, +-33}.  Each core computes banded
Gram blocks G[128k+r, 128k+c] (c in [0,161)) on the TensorEngine with
PSUM accumulation over its 2048-image shard; the host applies the C
weights to the diagonals and reduces across cores.

DMA-engine load balancing (measured on HW):
 - Full-128-partition transfers stripe port-aligned over all 16 SDMA
   engines at ~26.6 GB/s/engine, but engine 15 only sustains ~21.6 GB/s
   under concurrent load, so a uniform layout is engine-15-bound.
 - An N<128-partition transfer is dealt in k = (largest divisor of
   N <= 16) groups to engines 0..k-1 with cross-port writes at only
   ~13-16 GB/s/engine.
 The layout is therefore mildly ragged: all 128 partitions hold 14
 images (full-width transfers; engine 15 sheds 2 slots vs uniform-16),
 partitions 0-119 hold 2 more via one 120-row transfer (engines 0-14),
 partitions 0-15 one more via a 16-row transfer.  Slots a partition
 doesn't fill are zero in d and fall out of the Gram exactly.
"""

import numpy as np

_B = 16384
_H = 32
_W = 32
_P = _H * _W  # 1024 pixels
_NCORES = 8
_BSH = _B // _NCORES  # 2048 images per core
_TILE = 128
_BAND = 161  # 128 + max diagonal offset (33)

# Ragged slot layout (images per SBUF partition / DRAM row)
_NSLOT = 17  # max slots per row
_FREE = _NSLOT * _P  # 17408 f32 per partition
# row groups: [row0, row1, n_images]; 16*17 + 104*16 + 8*14 = 2048
_ROWGROUPS = [
    (0, 16, 17),
    (16, 120, 16),
    (120, 128, 14),
]
assert sum((r1 - r0) * n for r0, r1, n in _ROWGROUPS) == _BSH


def _block_ncols(k: int) -> int:
    return min(_BAND, _P - 128 * k)


_GRAM_COLS = sum(_block_ncols(k) for k in range(8))  # 7*161 + 128 = 1255


def _build_weights() -> np.ndarray:
    """[128, _GRAM_COLS] weights s.t. loss_sum = sum(W * gram_blocks)."""
    C = np.zeros((_P, _P), dtype=np.float64)
    offs = [(a, b) for a in range(3) for b in range(3) if (a, b) != (1, 1)]
    for a, b in offs:
        for i in range(_H - 2):
            for j in range(_W - 2):
                p = (i + a) * _W + (j + b)  # neighbor pixel
                q = (i + 1) * _W + (j + 1)  # center pixel
                C[p, p] += 1.0
                C[q, q] += 1.0
                C[p, q] -= 1.0
                C[q, p] -= 1.0
    W = np.zeros((_TILE, _GRAM_COLS), dtype=np.float64)
    off = 0
    for k in range(8):
        ncols = _block_ncols(k)
        for delta in (0, 1, 31, 32, 33):
            for r in range(_TILE):
                p = 128 * k + r
                q = p + delta
                c = r + delta
                if q >= _P or c >= ncols:
                    continue
                W[r, off + c] = C[p, q] * (1.0 if delta == 0 else 2.0)
        off += ncols
    return W


_WFULL = _build_weights()

_NC_CACHE = None


def _pack_shard(a: np.ndarray) -> np.ndarray:
    """[2048, 1024] f32 -> ragged-packed [128, _FREE]; pad cols unread."""
    Pk = np.empty((_TILE, _FREE), dtype=np.float32)
    img = 0
    for r0, r1, n in _ROWGROUPS:
        cnt = (r1 - r0) * n
        Pk[r0:r1, : n * _P] = a[img : img + cnt].reshape(r1 - r0, n * _P)
        img += cnt
    return Pk


def _build_nc():
    import concourse.bacc as bacc
    import concourse.mybir as mybir
    import concourse.tile as tile

    nc = bacc.Bacc()
    out_d = nc.dram_tensor("out", [_TILE, _FREE], mybir.dt.float32, kind="ExternalInput")
    lab_d = nc.dram_tensor("label", [_TILE, _FREE], mybir.dt.float32, kind="ExternalInput")
    gram_d = nc.dram_tensor(
        "gram", [_TILE, _GRAM_COLS], mybir.dt.float32, kind="ExternalOutput"
    )

    # DMA chunks: (row0, row1, col0, col1, slots-completed-after-this-chunk).
    # c0 goes first so engine 15 (full-width only) starts immediately; the
    # slower cross-port partial transfers run early-mid stream; the stream
    # ends on full-width 12KB descriptors with only slot 13's compute left.
    _CHUNKS = [
        (0, 128, 0, 4 * _P, (0, 1, 2, 3)),
        (0, 120, 14 * _P, 16 * _P, (14, 15)),
        (0, 16, 16 * _P, 17 * _P, (16,)),
        (0, 128, 4 * _P, 8 * _P, (4, 5, 6, 7)),
        (0, 128, 8 * _P, 11 * _P, (8, 9, 10)),
        (0, 128, 11 * _P, 14 * _P, (11, 12, 13)),
    ]

    def _slot_rows(tt):
        if tt < 14:
            return 128
        if tt < 16:
            return 120
        return 16

    with tile.TileContext(nc) as tc:
        with (
            tc.tile_pool(name="buf", bufs=1) as buf_pool,
            tc.tile_pool(name="ps", bufs=1, space="PSUM") as psum_pool,
        ):
            grams = []
            offs = []
            off = 0
            for k in range(8):
                ncols = _block_ncols(k)
                grams.append(
                    psum_pool.tile(
                        [_TILE, ncols], mybir.dt.float32, tag=f"g{k}", name=f"g{k}"
                    )
                )
                offs.append(off)
                off += ncols

            # persistent SBUF buffers: every chunk DMA can enqueue
            # immediately; no pool-slot rotation ever blocks the DMA stream.
            o = buf_pool.tile([_TILE, _FREE], mybir.dt.float32, tag="o", name="o")
            lb = buf_pool.tile([_TILE, _FREE], mybir.dt.float32, tag="l", name="l")
            d = buf_pool.tile([_TILE, _FREE], mybir.dt.bfloat16, tag="d", name="d")

            # zero d's ragged pad regions once, with quadrant-aligned
            # partition starts; later subs overwrite the real-data parts.
            nc.gpsimd.memset(d[96:128, 14 * _P : _FREE], 0.0)
            nc.gpsimd.memset(d[0:96, 16 * _P : _FREE], 0.0)

            def mm(k, base, first, last):
                ncols = _block_ncols(k)
                nc.tensor.matmul(
                    grams[k][:, :ncols],
                    lhsT=d[:, base + 128 * k : base + 128 * k + 128],
                    rhs=d[:, base + 128 * k : base + 128 * k + ncols],
                    start=first,
                    stop=last,
                )

            nslots_done = 0
            for r0, r1, c0, c1, slots in _CHUNKS:
                nc.sync.dma_start(out=o[r0:r1, c0:c1], in_=out_d[r0:r1, c0:c1])
                nc.scalar.dma_start(out=lb[r0:r1, c0:c1], in_=lab_d[r0:r1, c0:c1])
                for tt in slots:
                    base = tt * _P
                    nrow = _slot_rows(tt)
                    first = nslots_done == 0
                    last = nslots_done == _NSLOT - 1
                    if not last:
                        nc.vector.tensor_sub(
                            out=d[0:nrow, base : base + _P],
                            in0=o[0:nrow, base : base + _P],
                            in1=lb[0:nrow, base : base + _P],
                        )
                        for k in range(8):
                            mm(k, base, first, last)
                    else:
                        # final slot: high column half first so blocks 4-7
                        # stop early and their PSUM copies + output DMA
                        # overlap the low half's sub and matmuls
                        for h0, h1 in ((512, _P), (0, 512)):
                            nc.vector.tensor_sub(
                                out=d[0:nrow, base + h0 : base + h1],
                                in0=o[0:nrow, base + h0 : base + h1],
                                in1=lb[0:nrow, base + h0 : base + h1],
                            )
                        for k in (4, 5, 6, 7):
                            mm(k, base, first, last)
                    nslots_done += 1
            assert nslots_done == _NSLOT

            result = buf_pool.tile(
                [_TILE, _GRAM_COLS], mybir.dt.float32, tag="r", name="r"
            )
            base_last = 13 * _P
            for k in (4, 5, 6, 7):
                nc.vector.tensor_copy(
                    out=result[:, offs[k] : offs[k] + _block_ncols(k)], in_=grams[k][:]
                )
            nc.scalar.dma_start(out=gram_d[:, offs[4] :], in_=result[:, offs[4] :])
            for k in (0, 1, 2, 3):
                mm(k, base_last, False, True)
            for k in (0, 1, 2, 3):
                nc.vector.tensor_copy(
                    out=result[:, offs[k] : offs[k] + _block_ncols(k)], in_=grams[k][:]
                )
            nc.sync.dma_start(out=gram_d[:, : offs[4]], in_=result[:, : offs[4]])
    nc.finalize()
    return nc


def _run(out, label, trace=False):
    from concourse.bass_utils import run_bass_kernel_spmd

    global _NC_CACHE
    out = np.ascontiguousarray(np.asarray(out), dtype=np.float32).reshape(_B, _P)
    label = np.ascontiguousarray(np.asarray(label), dtype=np.float32).reshape(_B, _P)
    if _NC_CACHE is None:
        _NC_CACHE = _build_nc()
    in_maps = [
        {
            "out": _pack_shard(out[i * _BSH : (i + 1) * _BSH]),
            "label": _pack_shard(label[i * _BSH : (i + 1) * _BSH]),
        }
        for i in range(_NCORES)
    ]
    res = run_bass_kernel_spmd(
        _NC_CACHE, in_maps, core_ids=list(range(_NCORES)), trace=trace
    )
    total = 0.0
    for r in res.results:
        total += float((_WFULL * r["gram"].astype(np.float64)).sum())
    loss = total / (_B * 8 * (_H - 2) * (_W - 2))
    return np.asarray(np.float32(loss)), res


def kernel(out, label):
    loss, _ = _run(out, label, trace=False)
    return loss


# revision 13
# speedup vs baseline: 1.2784x; 1.0100x over previous
"""Contrast-depth MSE loss on 8 Trainium2 NeuronCores.

Math: with d = out - label (per image, 32x32 grid flattened to p in [0,1024)),
the loss is an exact quadratic form

    loss = sum_{p,q} C[p,q] * G[p,q] / (B*8*30*30),
    G[p,q] = sum_img d[img,p] * d[img,q]

where C (the contrast-depth-conv quadratic form) is supported on the
diagonals q-p in {0, +-1, +-31, +-32, +-33}.  Each core computes banded
Gram blocks G[128k+r, 128k+c] (c in [0,161)) on the TensorEngine with
PSUM accumulation over its 2048-image shard; the host applies the C
weights to the diagonals and reduces across cores.

DMA-engine load balancing (measured on HW):
 - Full-128-partition transfers stripe port-aligned over all 16 SDMA
   engines at ~26.6 GB/s/engine, but engine 15 only sustains ~21.6 GB/s
   under concurrent load, so a uniform layout is engine-15-bound.
 - An N<128-partition transfer is dealt in k = (largest divisor of
   N <= 16) groups to engines 0..k-1 with cross-port writes at only
   ~13-16 GB/s/engine.
 The layout is therefore mildly ragged: all 128 partitions hold 15
 images (full-width transfers; engine 15 sheds 1 slot vs uniform-16),
 partitions 0-63 hold 2 more via 60-row + 4-row transfers that engine
 15 never serves.  Slots a partition doesn't fill are zero in d and
 fall out of the Gram exactly.
"""

import numpy as np

_B = 16384
_H = 32
_W = 32
_P = _H * _W  # 1024 pixels
_NCORES = 8
_BSH = _B // _NCORES  # 2048 images per core
_TILE = 128
_BAND = 161  # 128 + max diagonal offset (33)

# Ragged slot layout (images per SBUF partition / DRAM row)
_NSLOT = 17  # max slots per row
_FREE = _NSLOT * _P  # 17408 f32 per partition
# row groups: [row0, row1, n_images]; 64*17 + 64*15 = 2048
_ROWGROUPS = [
    (0, 64, 17),
    (64, 128, 15),
]
assert sum((r1 - r0) * n for r0, r1, n in _ROWGROUPS) == _BSH


def _block_ncols(k: int) -> int:
    return min(_BAND, _P - 128 * k)


_GRAM_COLS = sum(_block_ncols(k) for k in range(8))  # 7*161 + 128 = 1255


def _build_weights() -> np.ndarray:
    """[128, _GRAM_COLS] weights s.t. loss_sum = sum(W * gram_blocks)."""
    C = np.zeros((_P, _P), dtype=np.float64)
    offs = [(a, b) for a in range(3) for b in range(3) if (a, b) != (1, 1)]
    for a, b in offs:
        for i in range(_H - 2):
            for j in range(_W - 2):
                p = (i + a) * _W + (j + b)  # neighbor pixel
                q = (i + 1) * _W + (j + 1)  # center pixel
                C[p, p] += 1.0
                C[q, q] += 1.0
                C[p, q] -= 1.0
                C[q, p] -= 1.0
    W = np.zeros((_TILE, _GRAM_COLS), dtype=np.float64)
    off = 0
    for k in range(8):
        ncols = _block_ncols(k)
        for delta in (0, 1, 31, 32, 33):
            for r in range(_TILE):
                p = 128 * k + r
                q = p + delta
                c = r + delta
                if q >= _P or c >= ncols:
                    continue
                W[r, off + c] = C[p, q] * (1.0 if delta == 0 else 2.0)
        off += ncols
    return W


_WFULL = _build_weights()

_NC_CACHE = None


def _pack_shard(a: np.ndarray) -> np.ndarray:
    """[2048, 1024] f32 -> ragged-packed [128, _FREE]; pad cols unread."""
    Pk = np.empty((_TILE, _FREE), dtype=np.float32)
    img = 0
    for r0, r1, n in _ROWGROUPS:
        cnt = (r1 - r0) * n
        Pk[r0:r1, : n * _P] = a[img : img + cnt].reshape(r1 - r0, n * _P)
        img += cnt
    return Pk


def _build_nc():
    import concourse.bacc as bacc
    import concourse.mybir as mybir
    import concourse.tile as tile

    nc = bacc.Bacc()
    out_d = nc.dram_tensor("out", [_TILE, _FREE], mybir.dt.float32, kind="ExternalInput")
    lab_d = nc.dram_tensor("label", [_TILE, _FREE], mybir.dt.float32, kind="ExternalInput")
    gram_d = nc.dram_tensor(
        "gram", [_TILE, _GRAM_COLS], mybir.dt.float32, kind="ExternalOutput"
    )

    # DMA chunks: (row0, row1, col0, col1, slots-completed-after-this-chunk).
    # c0 goes first so engine 15 (full-width only) starts immediately; the
    # slower cross-port partial transfers run early-mid stream; the stream
    # ends on single-slot full-width chunks so the DVE subs pipeline right
    # behind the arriving data and only slot 14's compute trails the stream.
    _CHUNKS = [
        (0, 128, 0, 4 * _P, (0, 1, 2, 3)),
        (0, 60, 15 * _P, 17 * _P, ()),
        (60, 64, 15 * _P, 17 * _P, (15, 16)),
        (0, 128, 4 * _P, 8 * _P, (4, 5, 6, 7)),
        (0, 128, 8 * _P, 11 * _P, (8, 9, 10)),
        (0, 128, 11 * _P, 12 * _P, (11,)),
        (0, 128, 12 * _P, 13 * _P, (12,)),
        (0, 128, 13 * _P, 14 * _P, (13,)),
        (0, 128, 14 * _P, 15 * _P, (14,)),
    ]

    def _slot_rows(tt):
        return 128 if tt < 15 else 64

    with tile.TileContext(nc) as tc:
        with (
            tc.tile_pool(name="buf", bufs=1) as buf_pool,
            tc.tile_pool(name="ps", bufs=1, space="PSUM") as psum_pool,
        ):
            grams = []
            offs = []
            off = 0
            for k in range(8):
                ncols = _block_ncols(k)
                grams.append(
                    psum_pool.tile(
                        [_TILE, ncols], mybir.dt.float32, tag=f"g{k}", name=f"g{k}"
                    )
                )
                offs.append(off)
                off += ncols

            # persistent SBUF buffers: every chunk DMA can enqueue
            # immediately; no pool-slot rotation ever blocks the DMA stream.
            o = buf_pool.tile([_TILE, _FREE], mybir.dt.float32, tag="o", name="o")
            lb = buf_pool.tile([_TILE, _FREE], mybir.dt.float32, tag="l", name="l")
            d = buf_pool.tile([_TILE, _FREE], mybir.dt.bfloat16, tag="d", name="d")

            # zero d's ragged pad region once (rows 64-127, slots 15-16)
            nc.gpsimd.memset(d[64:128, 15 * _P : _FREE], 0.0)

            def mm(k, base, first, last):
                ncols = _block_ncols(k)
                nc.tensor.matmul(
                    grams[k][:, :ncols],
                    lhsT=d[:, base + 128 * k : base + 128 * k + 128],
                    rhs=d[:, base + 128 * k : base + 128 * k + ncols],
                    start=first,
                    stop=last,
                )

            nslots_done = 0
            for r0, r1, c0, c1, slots in _CHUNKS:
                nc.sync.dma_start(out=o[r0:r1, c0:c1], in_=out_d[r0:r1, c0:c1])
                nc.scalar.dma_start(out=lb[r0:r1, c0:c1], in_=lab_d[r0:r1, c0:c1])
                for tt in slots:
                    base = tt * _P
                    nrow = _slot_rows(tt)
                    first = nslots_done == 0
                    last = nslots_done == _NSLOT - 1
                    if not last:
                        nc.vector.tensor_sub(
                            out=d[0:nrow, base : base + _P],
                            in0=o[0:nrow, base : base + _P],
                            in1=lb[0:nrow, base : base + _P],
                        )
                        for k in range(8):
                            mm(k, base, first, last)
                    else:
                        # final slot: high column half first so blocks 4-7
                        # stop early and their PSUM copies + output DMA
                        # overlap the low half's sub and matmuls
                        for h0, h1 in ((512, _P), (0, 512)):
                            nc.vector.tensor_sub(
                                out=d[0:nrow, base + h0 : base + h1],
                                in0=o[0:nrow, base + h0 : base + h1],
                                in1=lb[0:nrow, base + h0 : base + h1],
                            )
                        for k in (4, 5, 6, 7):
                            mm(k, base, first, last)
                    nslots_done += 1
            assert nslots_done == _NSLOT

            result = buf_pool.tile(
                [_TILE, _GRAM_COLS], mybir.dt.float32, tag="r", name="r"
            )
            base_last = 14 * _P
            for k in (4, 5, 6, 7):
                nc.vector.tensor_copy(
                    out=result[:, offs[k] : offs[k] + _block_ncols(k)], in_=grams[k][:]
                )
            nc.scalar.dma_start(out=gram_d[:, offs[4] :], in_=result[:, offs[4] :])
            for k in (0, 1, 2, 3):
                mm(k, base_last, False, True)
            for k in (0, 1, 2, 3):
                nc.vector.tensor_copy(
                    out=result[:, offs[k] : offs[k] + _block_ncols(k)], in_=grams[k][:]
                )
            nc.sync.dma_start(out=gram_d[:, : offs[4]], in_=result[:, : offs[4]])
    nc.finalize()
    return nc


def _run(out, label, trace=False):
    from concourse.bass_utils import run_bass_kernel_spmd

    global _NC_CACHE
    out = np.ascontiguousarray(np.asarray(out), dtype=np.float32).reshape(_B, _P)
    label = np.ascontiguousarray(np.asarray(label), dtype=np.float32).reshape(_B, _P)
    if _NC_CACHE is None:
        _NC_CACHE = _build_nc()
    in_maps = [
        {
            "out": _pack_shard(out[i * _BSH : (i + 1) * _BSH]),
            "label": _pack_shard(label[i * _BSH : (i + 1) * _BSH]),
        }
        for i in range(_NCORES)
    ]
    res = run_bass_kernel_spmd(
        _NC_CACHE, in_maps, core_ids=list(range(_NCORES)), trace=trace
    )
    total = 0.0
    for r in res.results:
        total += float((_WFULL * r["gram"].astype(np.float64)).sum())
    loss = total / (_B * 8 * (_H - 2) * (_W - 2))
    return np.asarray(np.float32(loss)), res


def kernel(out, label):
    loss, _ = _run(out, label, trace=False)
    return loss
